# revision 15
# baseline (speedup 1.0000x reference)
"""Trainium2 Bass kernel for nn_Attention (LayerNorm -> MHA -> out-proj).

Full (unsharded) inputs in, full output out. Internally shards across 8
NeuronCores as (batch b in 0..3) x (head-group g in 0..1): core c = 2*b + g
computes batch b, heads [g*8, g*8+8) of 16, producing a partial output
projection [2048, 1024]; the host sums the two group partials per batch and
adds b_out.

v3 design:
  Phase A: x arrives fp16; LayerNorm stats via bn_stats/bn_aggr (DVE) with
    the stats chain software-pipelined one tile ahead; xn = (x-mu)*rstd
    fused on ACT (Identity, per-partition scale/bias APs); PE transposes
    xn -> xnT; QKV projections on PE; PSUM evacuations split ACT/DVE.
  Attention (per head pair (2i, 2i+1), q in 512-col quarters):
    - K=64 score matmuls of the two heads run CONCURRENTLY in disjoint
      PE-array row halves (auto tile_position via base partitions 0/64).
    - exp split: head A on ACT (native Exp), head B on DVE via a
      Schraudolph bit-trick (s*C1+C2 cast to uint16 == fp16 bits of
      exp(s*SCALE)); every 16th key block's B-exp goes to ACT for balance.
    - PV accumulates over 16 key blocks (V carries a 65th ones-column
      producing the softmax denominator row); PV matmuls lag the scores
      by 2 iterations so the PE never waits on a fresh exp.
    - pv evacuation: ACT copies (unnormalized) into outT and the den row
      into a per-head staging row; normalization (reciprocal_approx_fast
      + gpsimd partition-broadcast + in-place multiply on outT) is
      deferred to the pair boundary where it overlaps the next pair's
      attention on otherwise-idle engines.
  Projection: out_partial[token, dim] = outT.T @ w_out_g^T, DMA out.
"""

import sys

if "/opt/trn_rl_repo" not in sys.path:
    sys.path.insert(0, "/opt/trn_rl_repo")

from contextlib import ExitStack

import numpy as np

import concourse.tile as tile
from concourse import bacc, mybir
from concourse.bass_utils import run_bass_kernel_spmd
from concourse.masks import make_identity

P = 128
N_TOK = 2048
DIM = 1024
HEADS_TOTAL = 16
H = 8  # heads per core
DH = 64
GI = H * DH  # 512, per-core inner size
INNER = HEADS_TOTAL * DH  # 1024
N_CORES = 8
SCALE = DH ** -0.5
EPS = 1e-5

AF = mybir.ActivationFunctionType
AX = mybir.AxisListType
ALU = mybir.AluOpType
f32 = mybir.dt.float32
fp16 = mybir.dt.float16
u16 = mybir.dt.uint16

# exp bit trick: fp16 bits of exp(s*SCALE) ~= round(s*EC1 + EC2)
EC1 = SCALE * 1024.0 * 1.4426950408889634
EC2 = 15360.0

_CACHE = {}


def build_nc(apply_gb=False):
    nc = bacc.Bacc("TRN2", target_bir_lowering=False, debug=False)
    x_d = nc.dram_tensor("x", [N_TOK, DIM], fp16, kind="ExternalInput").ap()
    wq_d = nc.dram_tensor("wq", [P, 8 * GI], fp16, kind="ExternalInput").ap()
    wk_d = nc.dram_tensor("wk", [P, 8 * GI], fp16, kind="ExternalInput").ap()
    wv_d = nc.dram_tensor("wv", [P, 8 * GI], fp16, kind="ExternalInput").ap()
    wo_d = nc.dram_tensor("wo", [4, P, DIM], fp16, kind="ExternalInput").ap()
    gb_d = None
    if apply_gb:
        gb_d = (nc.dram_tensor("gbc", [P, DIM], fp16, kind="ExternalInput").ap(),
                nc.dram_tensor("bbc", [P, DIM], fp16, kind="ExternalInput").ap())
    out_d = nc.dram_tensor("out", [N_TOK, DIM], f32, kind="ExternalOutput").ap()

    with tile.TileContext(nc) as tc:
        _body(nc, tc, x_d, wq_d, wk_d, wv_d, wo_d, gb_d, out_d)
    nc.compile()
    return nc


def _body(nc, tc, x_d, wq_d, wk_d, wv_d, wo_d, gb_d, out_d):
    apply_gb = gb_d is not None
    # ---- raw (whole-kernel) SBUF tensors ----
    ident = nc.alloc_sbuf_tensor("ident", [P, P], f32)
    make_identity(nc, ident[:, :])
    identh = nc.alloc_sbuf_tensor("identh", [P, P], fp16)
    nc.vector.tensor_copy(identh[:, :], ident[:, :])
    ones8f = nc.alloc_sbuf_tensor("ones8f", [P, H, 1], f32)
    nc.vector.memset(ones8f[:, :, :], 1.0)
    ones8r = nc.alloc_sbuf_tensor("ones8r", [P, H, 1], fp16)
    nc.vector.tensor_copy(ones8r[:, :, :], ones8f[:, :, :])
    epsb = nc.alloc_sbuf_tensor("epsb", [P, 1], f32)
    nc.vector.memset(epsb[:, :], EPS)

    QT = [nc.alloc_sbuf_tensor(f"qtt{p}", [P, N_TOK], fp16) for p in range(4)]
    KT = [nc.alloc_sbuf_tensor(f"ktt{p}", [P, N_TOK], fp16) for p in range(4)]
    V = nc.alloc_sbuf_tensor("vt", [P, 16, H, P], fp16)
    nc.vector.memset(V[:, :, :, :], 0.0)
    for t in range(16):
        nc.vector.tensor_copy(V[:, t, :, DH : DH + 1], ones8r[:, :, :])

    # ---- phase A: LayerNorm + transpose + QKV projections ----
    with tc.tile_pool(name="phW", bufs=1) as phW, \
         tc.tile_pool(name="phA", bufs=2) as phA, \
         tc.tile_pool(name="phAx", bufs=6) as phAx, \
         tc.tile_pool(name="phAs", bufs=4) as phAs, \
         tc.tile_pool(name="tpsum", bufs=2, space="PSUM") as tpsum, \
         tc.tile_pool(name="qpsum", bufs=4, space="PSUM") as qpsum:
        wq_sb = phW.tile([P, 8 * GI], fp16, tag="wq")
        nc.gpsimd.dma_start(wq_sb[:], wq_d[:])
        wk_sb = phW.tile([P, 8 * GI], fp16, tag="wk")
        nc.gpsimd.dma_start(wk_sb[:], wk_d[:])
        wv_sb = phW.tile([P, 8 * GI], fp16, tag="wv")
        nc.gpsimd.dma_start(wv_sb[:], wv_d[:])
        if apply_gb:
            gbc = phW.tile([P, DIM], fp16, tag="gbc")
            nc.gpsimd.dma_start(gbc[:], gb_d[0][:])
            bbc = phW.tile([P, DIM], fp16, tag="bbc")
            nc.gpsimd.dma_start(bbc[:], gb_d[1][:])

        n_stage = 4  # token stages
        stok = N_TOK // n_stage  # 512
        tpst = stok // P  # 4 token tiles per stage

        # software-pipelined LN stats: DMA + stats chain for tile t,
        # returning (xt, mv) handles consumed one tile later.
        def ln_stats(t):
            xt = phAx.tile([P, DIM], fp16, tag="x", name="x")
            xq = (nc.sync, nc.scalar)[t % 2]
            xq.dma_start(xt[:], x_d[t * P : (t + 1) * P, :])
            st = phAs.tile([P, 2, 6], f32, tag="st", name="st")
            nc.vector.bn_stats(st[:, 0, :], xt[:, 0:512])
            nc.vector.bn_stats(st[:, 1, :], xt[:, 512:1024])
            mv = phAs.tile([P, 4], f32, tag="mv", name="mv")
            nc.vector.bn_aggr(mv[:, 0:2], st[:, :, :])
            std, rstd, nmrs = mv[:, 2:3], mv[:, 3:4], mv[:, 1:2]
            nc.scalar.activation(std, mv[:, 1:2], AF.Sqrt, bias=epsb[:, :])
            nc.vector.reciprocal_approx_fast(rstd, std)
            # nmrs = -mu * rstd (overwrites var slot)
            nc.vector.scalar_tensor_tensor(nmrs, mv[:, 0:1], -1.0, rstd,
                                           op0=ALU.mult, op1=ALU.mult)
            return xt, mv

        pending = ln_stats(0)
        for q in range(n_stage):
            xnT = phA.tile([P, 8, stok], fp16, tag="xnt", name="xnt")
            for tt in range(tpst):
                t = q * tpst + tt
                xt, mv = pending
                if t + 1 < 16:
                    pending = ln_stats(t + 1)
                rstd, nmrs = mv[:, 3:4], mv[:, 1:2]
                xh = phAx.tile([P, DIM], fp16, tag="xh", name="xh")
                nc.scalar.activation(xh[:], xt[:], AF.Identity,
                                     bias=nmrs, scale=rstd)
                if apply_gb:
                    nc.vector.tensor_mul(xh[:], xh[:], gbc[:])
                    nc.vector.tensor_add(xh[:], xh[:], bbc[:])
                for d in range(8):
                    tp = tpsum.tile([P, P], fp16, tag="tp", name="tp")
                    nc.tensor.transpose(tp[:], xh[:, d * P : (d + 1) * P],
                                        identh[:, :])
                    dst = xnT[:, d, tt * P : (tt + 1) * P]
                    if d % 2 == 0:
                        nc.scalar.copy(dst, tp[:])
                    else:
                        nc.vector.tensor_copy(dst, tp[:])
            # Q^T / K^T pieces: [128 rows of head-features, stok tokens]
            for p in range(4):
                for wi, (wsb, dstT) in enumerate(((wq_sb, QT), (wk_sb, KT))):
                    ps = qpsum.tile([P, 512], f32, tag="qp", name="qp")
                    for d in range(8):
                        lo = d * GI + p * P
                        nc.tensor.matmul(ps[:, 0:stok], wsb[:, lo : lo + P],
                                         xnT[:, d, :],
                                         start=(d == 0), stop=(d == 7))
                    dst = dstT[p][:, q * stok : (q + 1) * stok]
                    if (p + wi) % 2 == 0:
                        nc.scalar.copy(dst, ps[:, 0:stok])
                    else:
                        nc.vector.tensor_copy(dst, ps[:, 0:stok])
            # V pieces: [128 tokens, 512 features]
            for tt in range(tpst):
                t = q * tpst + tt
                ps = qpsum.tile([P, 512], f32, tag="qp", name="qp")
                for d in range(8):
                    nc.tensor.matmul(ps[:], xnT[:, d, tt * P : (tt + 1) * P],
                                     wv_sb[:, d * GI : (d + 1) * GI],
                                     start=(d == 0), stop=(d == 7))
                nc.vector.tensor_copy(
                    V[:, t, :, 0:DH],
                    ps[:].rearrange("p (h w) -> p h w", w=DH))

    # ---- attention ----
    outT = [nc.alloc_sbuf_tensor(f"ott{p}", [P, N_TOK], fp16) for p in range(4)]
    with tc.tile_pool(name="wop", bufs=1) as wop:
        wo_sb = [wop.tile([P, DIM], fp16, tag=f"wo{p}", name=f"wo{p}")
                 for p in range(4)]
        for p in range(4):
            nc.sync.dma_start(wo_sb[p][:], wo_d[p])

        att_stack = ExitStack()
        attS = att_stack.enter_context(tc.tile_pool(name="attS", bufs=8))
        attN = att_stack.enter_context(tc.tile_pool(name="attN", bufs=4))
        spool = att_stack.enter_context(
            tc.tile_pool(name="spool", bufs=2, space="PSUM"))
        pvpool = att_stack.enter_context(
            tc.tile_pool(name="pvpool", bufs=4, space="PSUM"))

        norm_q = []  # deferred normalize multiplies, emitted mid-next-qq

        def emit_norm(i, hh, pv, bcs, q0):
            def f():
                r0, r1 = hh * DH, (hh + 1) * DH
                nc.vector.tensor_tensor(outT[i][r0:r1, q0 : q0 + 512],
                                        pv[0:DH, :], bcs[0:DH, :],
                                        op=ALU.mult)
            return f

        for i in range(4):
            hA, hB = 2 * i, 2 * i + 1
            for qq in range(4):
                q0 = qq * 512
                pvA = pvpool.tile([P, 512], f32, tag="pv", name="pv")
                pvB = pvpool.tile([P, 512], f32, tag="pv", name="pv")
                pend = []  # (kb, esA_ap, esB_ap) awaiting PV, lag 2
                for kb in range(16):
                    k0 = kb * P
                    sps = spool.tile([P, 1024], f32, tag="sp", name="sp")
                    # paired K=64 score matmuls (PE rows 0:63 / 64:127)
                    nc.tensor.matmul(sps[:, 0:512],
                                     KT[i][0:64, k0 : k0 + P],
                                     QT[i][0:64, q0 : q0 + 512],
                                     start=True, stop=True)
                    nc.tensor.matmul(sps[:, 512:1024],
                                     KT[i][64:128, k0 : k0 + P],
                                     QT[i][64:128, q0 : q0 + 512],
                                     start=True, stop=True)
                    esA = attS.tile([P, 512], fp16, tag="esA", name="esA")
                    nc.scalar.activation(esA[:], sps[:, 0:512], AF.Exp,
                                         scale=SCALE)
                    if kb % 16 == 15:  # rebalance: ACT takes this one
                        esB = attS.tile([P, 512], fp16, tag="esBa",
                                        name="esBa")
                        nc.scalar.activation(esB[:], sps[:, 512:1024],
                                             AF.Exp, scale=SCALE)
                        pBh = esB[:]
                    else:  # DVE bit-trick exp -> fp16 bits in uint16
                        esB = attS.tile([P, 512], u16, tag="esB", name="esB")
                        nc.vector.tensor_scalar(esB[:], sps[:, 512:1024],
                                                EC1, EC2,
                                                op0=ALU.mult, op1=ALU.add)
                        pBh = esB[:].bitcast(fp16)
                    if kb in (2, 4, 8, 12) and norm_q:
                        norm_q.pop(0)()
                    pend.append((kb, esA[:], pBh))
                    if len(pend) > 3:
                        pkb, pA, pB = pend.pop(0)
                        nc.tensor.matmul(pvA[:, :],
                                         V[:, pkb, hA, :], pA,
                                         start=(pkb == 0), stop=False)
                        nc.tensor.matmul(pvB[:, :],
                                         V[:, pkb, hB, :], pB,
                                         start=(pkb == 0), stop=False)
                for pkb, pA, pB in pend:
                    last = pkb == 15
                    nc.tensor.matmul(pvA[:, :], V[:, pkb, hA, :],
                                     pA, start=False, stop=last)
                    nc.tensor.matmul(pvB[:, :], V[:, pkb, hB, :],
                                     pB, start=False, stop=last)
                # normalization steps (den reciprocal+broadcast, then the
                # outT multiply) are deferred into the next qq's stream
                for hh, pv in ((0, pvA), (1, pvB)):
                    bcs = attN.tile([P, 512], f32, tag="bcs", name="bcs")

                    def recip_bcast(pv=pv, bcs=bcs):
                        rd = attN.tile([1, 512], f32, tag="ds", name="ds")
                        nc.scalar.copy(rd[0:1, :], pv[DH : DH + 1, :])
                        nc.vector.reciprocal_approx_fast(rd[0:1, :],
                                                         rd[0:1, :])
                        nc.gpsimd.partition_broadcast(bcs[0:DH, :],
                                                      rd[0:1, :], channels=DH)

                    norm_q.append(recip_bcast)
                    norm_q.append(emit_norm(i, hh, pv, bcs, q0))
        while norm_q:
            norm_q.pop(0)()
        att_stack.close()  # release attention SBUF/PSUM pools

        # ---- output projection ----
        with tc.tile_pool(name="proj", bufs=2) as proj, \
             tc.tile_pool(name="ppsum", bufs=2, space="PSUM") as ppsum:
            for t in range(16):
                pp = ppsum.tile([P, DIM], f32, tag="pp", name="pp")
                for nn in range(2):
                    cs = slice(nn * 512, (nn + 1) * 512)
                    for p in range(4):
                        nc.tensor.matmul(pp[:, cs],
                                         outT[p][:, t * P : (t + 1) * P],
                                         wo_sb[p][:, cs],
                                         start=(p == 0), stop=(p == 3))
                ob = proj.tile([P, DIM], f32, tag="ob", name="ob")
                if t % 2 == 0:
                    nc.scalar.copy(ob[:], pp[:])
                else:
                    nc.vector.tensor_copy(ob[:], pp[:])
                nc.sync.dma_start(out_d[t * P : (t + 1) * P, :], ob[:])


def _host_prep(x, ln_gamma, ln_beta, w_qkv, w_out, apply_gb):
    """Build per-core input maps."""

    def wchunks(w):  # w: [GI, DIM] rows=features -> [128, 8*512] lhsT chunks
        wt = np.ascontiguousarray(w.T, dtype=np.float16)  # [DIM, GI]
        return np.concatenate([wt[d * P : (d + 1) * P, :] for d in range(8)],
                              axis=1)

    in_maps = []
    for b in range(4):
        for g in range(2):
            lo, hi = g * GI, (g + 1) * GI
            m = {
                "x": np.ascontiguousarray(x[b], dtype=np.float16),
                "wq": wchunks(w_qkv[lo:hi, :]),
                "wk": wchunks(w_qkv[INNER + lo : INNER + hi, :]),
                "wv": wchunks(w_qkv[2 * INNER + lo : 2 * INNER + hi, :]),
                "wo": np.ascontiguousarray(
                    w_out[:, lo:hi].T.reshape(4, P, DIM), dtype=np.float16),
            }
            if apply_gb:
                m["gbc"] = np.ascontiguousarray(
                    np.broadcast_to(ln_gamma[None, :], (P, DIM)),
                    dtype=np.float16)
                m["bbc"] = np.ascontiguousarray(
                    np.broadcast_to(ln_beta[None, :], (P, DIM)),
                    dtype=np.float16)
            in_maps.append(m)
    return in_maps


def _run(inputs, trace=False):
    ln_gamma = np.asarray(inputs["ln_gamma"], dtype=np.float32)
    ln_beta = np.asarray(inputs["ln_beta"], dtype=np.float32)
    apply_gb = bool((ln_gamma != 1.0).any() or (ln_beta != 0.0).any())
    key = ("nc", apply_gb)
    if key not in _CACHE:
        _CACHE[key] = build_nc(apply_gb=apply_gb)
    nc = _CACHE[key]
    in_maps = _host_prep(inputs["x"], ln_gamma, ln_beta,
                         inputs["w_qkv"], inputs["w_out"], apply_gb)
    res = run_bass_kernel_spmd(nc, in_maps, list(range(N_CORES)), trace=trace)
    b_out = np.asarray(inputs["b_out"], dtype=np.float32)
    out = np.empty((4, N_TOK, DIM), dtype=np.float32)
    for b in range(4):
        out[b] = (res.results[2 * b]["out"] + res.results[2 * b + 1]["out"]
                  + b_out[None, :])
    return out, res


def kernel(**inputs):
    out, _ = _run(inputs, trace=False)
    return out


def kernel_profiled(**inputs):
    out, res = _run(inputs, trace=True)
    return out, res


# revision 16
# speedup vs baseline: 1.0087x; 1.0087x over previous
"""Trainium2 Bass kernel for nn_Attention (LayerNorm -> MHA -> out-proj).

Full (unsharded) inputs in, full output out. Internally shards across 8
NeuronCores as (batch b in 0..3) x (head-group g in 0..1): core c = 2*b + g
computes batch b, heads [g*8, g*8+8) of 16, producing a partial output
projection [2048, 1024]; the host sums the two group partials per batch and
adds b_out.

v3 design:
  Phase A: x arrives fp16; LayerNorm stats via bn_stats/bn_aggr (DVE) with
    the stats chain software-pipelined one tile ahead; xn = (x-mu)*rstd
    fused on ACT (Identity, per-partition scale/bias APs); PE transposes
    xn -> xnT; QKV projections on PE; PSUM evacuations split ACT/DVE.
  Attention (per head pair (2i, 2i+1), q in 512-col quarters):
    - K=64 score matmuls of the two heads run CONCURRENTLY in disjoint
      PE-array row halves (auto tile_position via base partitions 0/64).
    - exp split: head A on ACT (native Exp), head B on DVE via a
      Schraudolph bit-trick (s*C1+C2 cast to uint16 == fp16 bits of
      exp(s*SCALE)); every 16th key block's B-exp goes to ACT for balance.
    - PV accumulates over 16 key blocks (V carries a 65th ones-column
      producing the softmax denominator row); PV matmuls lag the scores
      by 2 iterations so the PE never waits on a fresh exp.
    - pv evacuation: ACT copies (unnormalized) into outT and the den row
      into a per-head staging row; normalization (reciprocal_approx_fast
      + gpsimd partition-broadcast + in-place multiply on outT) is
      deferred to the pair boundary where it overlaps the next pair's
      attention on otherwise-idle engines.
  Projection: out_partial[token, dim] = outT.T @ w_out_g^T, DMA out.
"""

import sys

if "/opt/trn_rl_repo" not in sys.path:
    sys.path.insert(0, "/opt/trn_rl_repo")

from contextlib import ExitStack

import numpy as np

import concourse.tile as tile
from concourse import bacc, mybir
from concourse.bass_utils import run_bass_kernel_spmd
from concourse.masks import make_identity

P = 128
N_TOK = 2048
DIM = 1024
HEADS_TOTAL = 16
H = 8  # heads per core
DH = 64
GI = H * DH  # 512, per-core inner size
INNER = HEADS_TOTAL * DH  # 1024
N_CORES = 8
SCALE = DH ** -0.5
EPS = 1e-5

AF = mybir.ActivationFunctionType
AX = mybir.AxisListType
ALU = mybir.AluOpType
f32 = mybir.dt.float32
fp16 = mybir.dt.float16
u16 = mybir.dt.uint16

# exp bit trick: fp16 bits of exp(s*SCALE) ~= round(s*EC1 + EC2)
EC1 = SCALE * 1024.0 * 1.4426950408889634
EC2 = 15360.0

_CACHE = {}


def build_nc(apply_gb=False):
    nc = bacc.Bacc("TRN2", target_bir_lowering=False, debug=False)
    x_d = nc.dram_tensor("x", [N_TOK, DIM], fp16, kind="ExternalInput").ap()
    wq_d = nc.dram_tensor("wq", [P, 8 * GI], fp16, kind="ExternalInput").ap()
    wk_d = nc.dram_tensor("wk", [P, 8 * GI], fp16, kind="ExternalInput").ap()
    wv_d = nc.dram_tensor("wv", [P, 8 * GI], fp16, kind="ExternalInput").ap()
    wo_d = nc.dram_tensor("wo", [4, P, DIM], fp16, kind="ExternalInput").ap()
    gb_d = None
    if apply_gb:
        gb_d = (nc.dram_tensor("gbc", [P, DIM], fp16, kind="ExternalInput").ap(),
                nc.dram_tensor("bbc", [P, DIM], fp16, kind="ExternalInput").ap())
    out_d = nc.dram_tensor("out", [N_TOK, DIM], f32, kind="ExternalOutput").ap()

    with tile.TileContext(nc) as tc:
        _body(nc, tc, x_d, wq_d, wk_d, wv_d, wo_d, gb_d, out_d)
    nc.compile()
    return nc


def _body(nc, tc, x_d, wq_d, wk_d, wv_d, wo_d, gb_d, out_d):
    apply_gb = gb_d is not None
    # ---- raw (whole-kernel) SBUF tensors ----
    ident = nc.alloc_sbuf_tensor("ident", [P, P], f32)
    make_identity(nc, ident[:, :])
    identh = nc.alloc_sbuf_tensor("identh", [P, P], fp16)
    nc.vector.tensor_copy(identh[:, :], ident[:, :])
    ones8f = nc.alloc_sbuf_tensor("ones8f", [P, H, 1], f32)
    nc.vector.memset(ones8f[:, :, :], 1.0)
    ones8r = nc.alloc_sbuf_tensor("ones8r", [P, H, 1], fp16)
    nc.vector.tensor_copy(ones8r[:, :, :], ones8f[:, :, :])
    epsb = nc.alloc_sbuf_tensor("epsb", [P, 1], f32)
    nc.vector.memset(epsb[:, :], EPS)

    QT = [nc.alloc_sbuf_tensor(f"qtt{p}", [P, N_TOK], fp16) for p in range(4)]
    KT = [nc.alloc_sbuf_tensor(f"ktt{p}", [P, N_TOK], fp16) for p in range(4)]
    V = nc.alloc_sbuf_tensor("vt", [P, 16, H, DH + 1], fp16)
    for t in range(16):
        nc.vector.tensor_copy(V[:, t, :, DH : DH + 1], ones8r[:, :, :])

    # ---- phase A: LayerNorm + transpose + QKV projections ----
    with tc.tile_pool(name="phW", bufs=1) as phW, \
         tc.tile_pool(name="phA", bufs=2) as phA, \
         tc.tile_pool(name="phAx", bufs=6) as phAx, \
         tc.tile_pool(name="phAs", bufs=4) as phAs, \
         tc.tile_pool(name="tpsum", bufs=2, space="PSUM") as tpsum, \
         tc.tile_pool(name="qpsum", bufs=4, space="PSUM") as qpsum:
        wq_sb = phW.tile([P, 8 * GI], fp16, tag="wq")
        nc.gpsimd.dma_start(wq_sb[:], wq_d[:])
        wk_sb = phW.tile([P, 8 * GI], fp16, tag="wk")
        nc.gpsimd.dma_start(wk_sb[:], wk_d[:])
        wv_sb = phW.tile([P, 8 * GI], fp16, tag="wv")
        nc.gpsimd.dma_start(wv_sb[:], wv_d[:])
        if apply_gb:
            gbc = phW.tile([P, DIM], fp16, tag="gbc")
            nc.gpsimd.dma_start(gbc[:], gb_d[0][:])
            bbc = phW.tile([P, DIM], fp16, tag="bbc")
            nc.gpsimd.dma_start(bbc[:], gb_d[1][:])

        n_stage = 4  # token stages
        stok = N_TOK // n_stage  # 512
        tpst = stok // P  # 4 token tiles per stage

        # software-pipelined LN stats: DMA + stats chain for tile t,
        # returning (xt, mv) handles consumed one tile later.
        def ln_stats(t):
            xt = phAx.tile([P, DIM], fp16, tag="x", name="x")
            xq = (nc.sync, nc.scalar)[t % 2]
            xq.dma_start(xt[:], x_d[t * P : (t + 1) * P, :])
            st = phAs.tile([P, 2, 6], f32, tag="st", name="st")
            nc.vector.bn_stats(st[:, 0, :], xt[:, 0:512])
            nc.vector.bn_stats(st[:, 1, :], xt[:, 512:1024])
            mv = phAs.tile([P, 4], f32, tag="mv", name="mv")
            nc.vector.bn_aggr(mv[:, 0:2], st[:, :, :])
            std, rstd, nmrs = mv[:, 2:3], mv[:, 3:4], mv[:, 1:2]
            nc.scalar.activation(std, mv[:, 1:2], AF.Sqrt, bias=epsb[:, :])
            nc.vector.reciprocal_approx_fast(rstd, std)
            # nmrs = -mu * rstd (overwrites var slot)
            nc.vector.scalar_tensor_tensor(nmrs, mv[:, 0:1], -1.0, rstd,
                                           op0=ALU.mult, op1=ALU.mult)
            return xt, mv

        pending = ln_stats(0)
        for q in range(n_stage):
            xnT = phA.tile([P, 8, stok], fp16, tag="xnt", name="xnt")
            for tt in range(tpst):
                t = q * tpst + tt
                xt, mv = pending
                if t + 1 < 16:
                    pending = ln_stats(t + 1)
                rstd, nmrs = mv[:, 3:4], mv[:, 1:2]
                xh = phAx.tile([P, DIM], fp16, tag="xh", name="xh")
                nc.scalar.activation(xh[:], xt[:], AF.Identity,
                                     bias=nmrs, scale=rstd)
                if apply_gb:
                    nc.vector.tensor_mul(xh[:], xh[:], gbc[:])
                    nc.vector.tensor_add(xh[:], xh[:], bbc[:])
                for d in range(8):
                    tp = tpsum.tile([P, P], fp16, tag="tp", name="tp")
                    nc.tensor.transpose(tp[:], xh[:, d * P : (d + 1) * P],
                                        identh[:, :])
                    dst = xnT[:, d, tt * P : (tt + 1) * P]
                    if d % 2 == 0:
                        nc.scalar.copy(dst, tp[:])
                    else:
                        nc.vector.tensor_copy(dst, tp[:])
            # Q^T / K^T pieces: [128 rows of head-features, stok tokens]
            for p in range(4):
                for wi, (wsb, dstT) in enumerate(((wq_sb, QT), (wk_sb, KT))):
                    ps = qpsum.tile([P, 512], f32, tag="qp", name="qp")
                    for d in range(8):
                        lo = d * GI + p * P
                        nc.tensor.matmul(ps[:, 0:stok], wsb[:, lo : lo + P],
                                         xnT[:, d, :],
                                         start=(d == 0), stop=(d == 7))
                    dst = dstT[p][:, q * stok : (q + 1) * stok]
                    if (p + wi) % 2 == 0:
                        nc.scalar.copy(dst, ps[:, 0:stok])
                    else:
                        nc.vector.tensor_copy(dst, ps[:, 0:stok])
            # V pieces: [128 tokens, 512 features]
            for tt in range(tpst):
                t = q * tpst + tt
                ps = qpsum.tile([P, 512], f32, tag="qp", name="qp")
                for d in range(8):
                    nc.tensor.matmul(ps[:], xnT[:, d, tt * P : (tt + 1) * P],
                                     wv_sb[:, d * GI : (d + 1) * GI],
                                     start=(d == 0), stop=(d == 7))
                nc.vector.tensor_copy(
                    V[:, t, :, 0:DH],
                    ps[:].rearrange("p (h w) -> p h w", w=DH))

    # ---- attention ----
    outT = [nc.alloc_sbuf_tensor(f"ott{p}", [P, N_TOK], fp16) for p in range(4)]
    with tc.tile_pool(name="wop", bufs=1) as wop:
        wo_sb = [wop.tile([P, DIM], fp16, tag=f"wo{p}", name=f"wo{p}")
                 for p in range(4)]
        for p in range(4):
            nc.sync.dma_start(wo_sb[p][:], wo_d[p])

        att_stack = ExitStack()
        attS = att_stack.enter_context(tc.tile_pool(name="attS", bufs=8))
        attN = att_stack.enter_context(tc.tile_pool(name="attN", bufs=4))
        spool = att_stack.enter_context(
            tc.tile_pool(name="spool", bufs=2, space="PSUM"))
        pvpool = att_stack.enter_context(
            tc.tile_pool(name="pvpool", bufs=4, space="PSUM"))

        norm_q = []  # deferred normalize multiplies, emitted mid-next-qq

        def emit_norm(i, hh, pv, bcs, q0):
            def f():
                r0, r1 = hh * DH, (hh + 1) * DH
                nc.vector.tensor_tensor(outT[i][r0:r1, q0 : q0 + 512],
                                        pv[0:DH, :], bcs[0:DH, :],
                                        op=ALU.mult)
            return f

        for i in range(4):
            hA, hB = 2 * i, 2 * i + 1
            for qq in range(4):
                q0 = qq * 512
                pvA = pvpool.tile([P, 512], f32, tag="pv", name="pv")
                pvB = pvpool.tile([P, 512], f32, tag="pv", name="pv")
                pend = []  # (kb, esA_ap, esB_ap) awaiting PV, lag 2
                for kb in range(16):
                    k0 = kb * P
                    sps = spool.tile([P, 1024], f32, tag="sp", name="sp")
                    # paired K=64 score matmuls (PE rows 0:63 / 64:127)
                    nc.tensor.matmul(sps[:, 0:512],
                                     KT[i][0:64, k0 : k0 + P],
                                     QT[i][0:64, q0 : q0 + 512],
                                     start=True, stop=True)
                    nc.tensor.matmul(sps[:, 512:1024],
                                     KT[i][64:128, k0 : k0 + P],
                                     QT[i][64:128, q0 : q0 + 512],
                                     start=True, stop=True)
                    esA = attS.tile([P, 512], fp16, tag="esA", name="esA")
                    nc.scalar.activation(esA[:], sps[:, 0:512], AF.Exp,
                                         scale=SCALE)
                    if kb % 16 == 15:  # rebalance: ACT takes this one
                        esB = attS.tile([P, 512], fp16, tag="esBa",
                                        name="esBa")
                        nc.scalar.activation(esB[:], sps[:, 512:1024],
                                             AF.Exp, scale=SCALE)
                        pBh = esB[:]
                    else:  # DVE bit-trick exp -> fp16 bits in uint16
                        esB = attS.tile([P, 512], u16, tag="esB", name="esB")
                        nc.vector.tensor_scalar(esB[:], sps[:, 512:1024],
                                                EC1, EC2,
                                                op0=ALU.mult, op1=ALU.add)
                        pBh = esB[:].bitcast(fp16)
                    if kb in (2, 4, 8, 12) and norm_q:
                        norm_q.pop(0)()
                    pend.append((kb, esA[:], pBh))
                    if len(pend) > 3:
                        pkb, pA, pB = pend.pop(0)
                        nc.tensor.matmul(pvA[0 : DH + 1, :],
                                         V[:, pkb, hA, :], pA,
                                         start=(pkb == 0), stop=False)
                        nc.tensor.matmul(pvB[0 : DH + 1, :],
                                         V[:, pkb, hB, :], pB,
                                         start=(pkb == 0), stop=False)
                for pkb, pA, pB in pend:
                    last = pkb == 15
                    nc.tensor.matmul(pvA[0 : DH + 1, :], V[:, pkb, hA, :],
                                     pA, start=False, stop=last)
                    nc.tensor.matmul(pvB[0 : DH + 1, :], V[:, pkb, hB, :],
                                     pB, start=False, stop=last)
                # normalization steps (den reciprocal+broadcast, then the
                # outT multiply) are deferred into the next qq's stream
                for hh, pv in ((0, pvA), (1, pvB)):
                    bcs = attN.tile([P, 512], f32, tag="bcs", name="bcs")

                    def recip_bcast(pv=pv, bcs=bcs):
                        rd = attN.tile([1, 512], f32, tag="ds", name="ds")
                        nc.scalar.copy(rd[0:1, :], pv[DH : DH + 1, :])
                        nc.vector.reciprocal_approx_fast(rd[0:1, :],
                                                         rd[0:1, :])
                        nc.gpsimd.partition_broadcast(bcs[0:DH, :],
                                                      rd[0:1, :], channels=DH)

                    norm_q.append(recip_bcast)
                    norm_q.append(emit_norm(i, hh, pv, bcs, q0))
        while norm_q:
            norm_q.pop(0)()
        att_stack.close()  # release attention SBUF/PSUM pools

        # ---- output projection ----
        with tc.tile_pool(name="proj", bufs=2) as proj, \
             tc.tile_pool(name="ppsum", bufs=2, space="PSUM") as ppsum:
            for t in range(16):
                pp = ppsum.tile([P, DIM], f32, tag="pp", name="pp")
                for nn in range(2):
                    cs = slice(nn * 512, (nn + 1) * 512)
                    for p in range(4):
                        nc.tensor.matmul(pp[:, cs],
                                         outT[p][:, t * P : (t + 1) * P],
                                         wo_sb[p][:, cs],
                                         start=(p == 0), stop=(p == 3))
                ob = proj.tile([P, DIM], f32, tag="ob", name="ob")
                if t % 2 == 0:
                    nc.scalar.copy(ob[:], pp[:])
                else:
                    nc.vector.tensor_copy(ob[:], pp[:])
                nc.sync.dma_start(out_d[t * P : (t + 1) * P, :], ob[:])


def _host_prep(x, ln_gamma, ln_beta, w_qkv, w_out, apply_gb):
    """Build per-core input maps."""

    def wchunks(w):  # w: [GI, DIM] rows=features -> [128, 8*512] lhsT chunks
        wt = np.ascontiguousarray(w.T, dtype=np.float16)  # [DIM, GI]
        return np.concatenate([wt[d * P : (d + 1) * P, :] for d in range(8)],
                              axis=1)

    in_maps = []
    for b in range(4):
        for g in range(2):
            lo, hi = g * GI, (g + 1) * GI
            m = {
                "x": np.ascontiguousarray(x[b], dtype=np.float16),
                "wq": wchunks(w_qkv[lo:hi, :]),
                "wk": wchunks(w_qkv[INNER + lo : INNER + hi, :]),
                "wv": wchunks(w_qkv[2 * INNER + lo : 2 * INNER + hi, :]),
                "wo": np.ascontiguousarray(
                    w_out[:, lo:hi].T.reshape(4, P, DIM), dtype=np.float16),
            }
            if apply_gb:
                m["gbc"] = np.ascontiguousarray(
                    np.broadcast_to(ln_gamma[None, :], (P, DIM)),
                    dtype=np.float16)
                m["bbc"] = np.ascontiguousarray(
                    np.broadcast_to(ln_beta[None, :], (P, DIM)),
                    dtype=np.float16)
            in_maps.append(m)
    return in_maps


def _run(inputs, trace=False):
    ln_gamma = np.asarray(inputs["ln_gamma"], dtype=np.float32)
    ln_beta = np.asarray(inputs["ln_beta"], dtype=np.float32)
    apply_gb = bool((ln_gamma != 1.0).any() or (ln_beta != 0.0).any())
    key = ("nc", apply_gb)
    if key not in _CACHE:
        _CACHE[key] = build_nc(apply_gb=apply_gb)
    nc = _CACHE[key]
    in_maps = _host_prep(inputs["x"], ln_gamma, ln_beta,
                         inputs["w_qkv"], inputs["w_out"], apply_gb)
    res = run_bass_kernel_spmd(nc, in_maps, list(range(N_CORES)), trace=trace)
    b_out = np.asarray(inputs["b_out"], dtype=np.float32)
    out = np.empty((4, N_TOK, DIM), dtype=np.float32)
    for b in range(4):
        out[b] = (res.results[2 * b]["out"] + res.results[2 * b + 1]["out"]
                  + b_out[None, :])
    return out, res


def kernel(**inputs):
    out, _ = _run(inputs, trace=False)
    return out


def kernel_profiled(**inputs):
    out, res = _run(inputs, trace=True)
    return out, res
